# revision 1
# baseline (speedup 1.0000x reference)
"""CommutatorConv2d kernel for Trainium2 (Bass/Tile), 8-core data-parallel.

Math: the reference's commutator/anticommutator conv reduces exactly to a
single-channel 3x3 conv on the channel-summed input:

    out[b] = T @ xs[b] @ A + Bm @ xs[b] @ T + bias,   xs = x.sum(axis=1)

where T is the 128x128 tridiagonal-ones matrix and A, Bm are tridiagonal
matrices built from K's column/row sums scaled by (lambda_c +/- lambda_a).

v5 (all bf16; harness gate is 2e-2, this lands ~5e-3):
- One fused DRAM tensor per core: each partition row is
  [cmat row (A|T|BmT|I|bias-bits, 514 cols) | batch1 (32*128) | batch0].
  The constants ride inside the sync ring's FIRST piece, so they cost
  zero extra descriptors (HWDGE descriptor generation, ~15ns/desc/queue,
  is the stream bottleneck) and land before any compute needs them.
- Pieces of 16/12/4 channels per batch (4KB/3KB/1KB descriptors);
  batch 1 on the sync ring, batch 0 on the scalar ring.
- PE folds each batch's 16ch piece as 4 accumulating N=512 identity
  matmuls after a junk-matmul warmup block that ramps the PE out of its
  low/mid p-state; ACT evacuates the [128,512] stacks; the DVE (bf16 2x)
  trees the 12ch/4ch pieces, finishes the stacks, and combines into xs.
- Sandwich: uv = xs.T @ [T|BmT]; out = uv1.T @ A + uv2.T @ T in PSUM;
  ACT fuses the bias on PSUM evacuation. Stores are split into
  half-partition DMAs across both rings (halves descriptor-gen time).
"""

import numpy as np

B, C, H, W = 16, 32, 128, 128
N_CORES = 8
B_LOC = B // N_CORES

PIECE_B1 = (16, 12, 4)  # sync ring: head piece carries cmat too
PIECE_B0 = (16, 12, 4)  # scalar ring: symmetric, so the DGE head-of-line
# race (which ring's stream starts ~2us late is random) hurts either batch
# equally mildly — both tails end in a short 4ch tree
CMCOLS = 4 * W + 2
N_JUNK = 8

_PROGRAM = None
LAST_RESULTS = None


def _build_program():
    import concourse.mybir as mybir
    from concourse import bacc
    from concourse.bass import MemorySpace
    from concourse.tile import TileContext

    bf16 = mybir.dt.bfloat16
    f32 = mybir.dt.float32
    nc = bacc.Bacc(
        "TRN2", target_bir_lowering=False, debug=False, num_devices=N_CORES
    )

    ncols = CMCOLS + 2 * C * W
    xc_dram = nc.dram_tensor("xc", (H, ncols), bf16, kind="ExternalInput")
    # output 2x-packed: partition p holds rows 2p and 2p+1 -> 64-descriptor
    # stores instead of 128 (descriptor gen is ~13.5ns each)
    out_dram = nc.dram_tensor(
        "out", (H // 2, B_LOC, 2, W), f32, kind="ExternalOutput"
    )

    xc_ap = xc_dram.ap()
    out_ap = out_dram.ap()

    def col(b, c):
        # start column of channel c of batch b in the fused tensor
        # (batch 1 first, then batch 0)
        return CMCOLS + (1 - b) * C * W + c * W

    with TileContext(nc) as tc:
        with (
            tc.tile_pool(name="xpool", bufs=1) as xpool,
            tc.tile_pool(name="spool", bufs=1) as spool,
            tc.tile_pool(name="psum", bufs=1, space=MemorySpace.PSUM) as ppool,
        ):
            # PE warmup scratch (zeroed on gpsimd, otherwise idle)
            scratch = spool.tile([H, 5 * W], bf16, tag="scratch")
            nc.gpsimd.memset(scratch, 0.0)

            # sync ring: [cmat + b1p0] then b1p1, b1p2, then store halves;
            # scalar ring: b0 pieces then store halves
            head = xpool.tile([H, CMCOLS + PIECE_B1[0] * W], bf16, tag="head")
            nc.sync.dma_start(out=head, in_=xc_ap[:, 0 : CMCOLS + PIECE_B1[0] * W])
            cm_sb = head[:, 0:CMCOLS]
            a_sb = cm_sb[:, 0:W]
            t_sb = cm_sb[:, W : 2 * W]
            tbm_sb = cm_sb[:, W : 3 * W]  # [T | BmT]
            i_sb = cm_sb[:, 3 * W : 4 * W]
            bias_sb = cm_sb[:, 4 * W : 4 * W + 2].bitcast(f32)

            tiles = {(1, 0): head[:, CMCOLS : CMCOLS + PIECE_B1[0] * W]}
            for b, eng, pieces in (
                (1, nc.sync, PIECE_B1),
                (0, nc.scalar, PIECE_B0),
            ):
                c0 = 0
                for p, nch in enumerate(pieces):
                    if (b, p) not in tiles:
                        xq = xpool.tile([H, nch * W], bf16, tag=f"x{b}_{p}")
                        eng.dma_start(
                            out=xq, in_=xc_ap[:, col(b, c0) : col(b, c0 + nch)]
                        )
                        tiles[(b, p)] = xq
                    c0 += nch

            # ---- PE: warmup junk quads, then each batch's 16ch piece as
            # 4 accumulating N=512 identity matmuls ----
            junk_psum = ppool.tile([H, 4 * W], f32, tag="junk")
            for j in range(N_JUNK):
                nc.tensor.matmul(
                    junk_psum,
                    scratch[:, 0:W],
                    scratch[:, W : 5 * W],
                    start=True,
                    stop=True,
                    skip_group_check=True,
                )

            fold_psum = {}
            for b, nq in ((1, PIECE_B1[0] // 4), (0, PIECE_B0[0] // 4)):
                psum = ppool.tile([H, 4 * W], f32, tag=f"fold{b}")
                xq = tiles[(b, 0)]
                for q in range(nq):
                    nc.tensor.matmul(
                        psum,
                        i_sb,
                        xq[:, q * 4 * W : (q + 1) * 4 * W],
                        start=(q == 0),
                        stop=(q == nq - 1),
                        skip_group_check=True,
                    )
                fold_psum[b] = psum

            # ACT evacuates fold PSUMs to bf16 (mid-stream, hidden)
            p0_sb = {}
            for b in (1, 0):
                sb = spool.tile([H, 4 * W], bf16, tag=f"p0_{b}")
                nc.scalar.copy(sb, fold_psum[b])
                p0_sb[b] = sb

            # ---- DVE (ready-order): 12ch trees, then per-batch finish +
            # 4ch tail tree + combines ----
            def tree(ap_tile, nelem):
                n = nelem
                while n > W and n % 2 == 0 and (n // 2) % W == 0:
                    n //= 2
                    nc.vector.tensor_add(
                        ap_tile[:, :n], ap_tile[:, :n], ap_tile[:, n : 2 * n]
                    )
                if n == 3 * W:
                    nc.vector.tensor_add(
                        ap_tile[:, :W], ap_tile[:, :W], ap_tile[:, W : 2 * W]
                    )
                    nc.vector.tensor_add(
                        ap_tile[:, :W], ap_tile[:, :W], ap_tile[:, 2 * W : 3 * W]
                    )

            # batch 1's chain runs on the DVE; batch 0's stack-finish and
            # combines run on the otherwise-idle gpsimd (Pool) engine so the
            # two tails proceed in parallel instead of convoying on the DVE
            xs = {}
            tree(tiles[(1, 1)], 12 * W)
            tree(tiles[(0, 1)], 12 * W)
            pp1 = p0_sb[1]
            tree(pp1, 4 * W)  # finish PE stack: 512 -> 128
            # merge the 12ch partial before the late 4ch piece lands, so
            # only tree4 + one add sit behind the last DMA
            nc.vector.tensor_add(pp1[:, :W], pp1[:, :W], tiles[(1, 1)][:, :W])

            pp0 = p0_sb[0]
            n = 4 * W
            while n > W:
                n //= 2
                nc.gpsimd.tensor_add(pp0[:, :n], pp0[:, :n], pp0[:, n : 2 * n])
            nc.gpsimd.tensor_add(pp0[:, :W], pp0[:, :W], tiles[(0, 1)][:, :W])

            # both 4ch tail trees back-to-back on the DVE the moment the
            # last pieces land; b0's goes FIRST so its (slower) Pool final
            # merge starts earliest, converging with b1's DVE final merge
            tree(tiles[(0, 2)], 4 * W)
            tree(tiles[(1, 2)], 4 * W)
            nc.gpsimd.tensor_add(pp0[:, :W], pp0[:, :W], tiles[(0, 2)][:, :W])
            nc.vector.tensor_add(pp1[:, :W], pp1[:, :W], tiles[(1, 2)][:, :W])
            xs[1] = pp1[:, :W]
            xs[0] = pp0[:, :W]

            # ---- sandwich per batch (b1 first). stage2 uses stride-2 lhsT
            # slices so each PSUM partition p collects output rows 2p and
            # 2p+1 -> the store needs only 64 descriptors. ----
            H2 = H // 2
            uv_psum, uv_sb, o_psum, o_sb = {}, {}, {}, {}
            # phase 1: uv matmuls in column halves (same stationary xs) so
            # the low half's PSUM evacuation starts before the high half's
            # matmul finishes
            for b in (1, 0):
                uvp = ppool.tile([H, 2 * W], f32, tag=f"uv{b}p")
                uv_psum[b] = uvp
                nc.tensor.matmul(
                    uvp[:, 0:W], xs[b], t_sb, start=True, stop=True,
                    skip_group_check=True,
                )
                nc.tensor.matmul(
                    uvp[:, W : 2 * W], xs[b], cm_sb[:, 2 * W : 3 * W],
                    start=True, stop=True, skip_group_check=True,
                )
            # phase 2: halves evacuate concurrently per batch — ACT takes
            # the low half (gates stage2's first matmul), DVE the high half
            for b in (1, 0):
                uvs = spool.tile([H, 2 * W], bf16, tag=f"uv{b}")
                uv_sb[b] = uvs
                nc.scalar.copy(uv_sb[b][:, 0:W], uv_psum[b][:, 0:W])
                nc.vector.tensor_copy(
                    uv_sb[b][:, W : 2 * W], uv_psum[b][:, W : 2 * W]
                )
            # phase 3: stage2 (4 stride-2 matmuls per batch)
            for b in (1, 0):
                op = ppool.tile([H2, 2 * W], f32, tag=f"o{b}p")
                o_psum[b] = op
                for par in (0, 1):
                    dst = o_psum[b][:, par * W : (par + 1) * W]
                    nc.tensor.matmul(
                        dst, uv_sb[b][:, par : W : 2], a_sb,
                        start=True, stop=False, skip_group_check=True,
                    )
                    nc.tensor.matmul(
                        dst, uv_sb[b][:, W + par : 2 * W : 2], t_sb,
                        start=False, stop=True, skip_group_check=True,
                    )
            # phase 4+5: bias evac then half-split stores
            for b in (1, 0):
                osb = spool.tile([H2, 2 * W], f32, tag=f"o{b}")
                o_sb[b] = osb
                # evacuate per parity half: each depends only on its own
                # two stage2 matmuls, so the store launches earlier
                nc.scalar.add(
                    osb[:, 0:W], o_psum[b][:, 0:W], add=bias_sb[0:H2, :]
                )
                nc.scalar.add(
                    osb[:, W : 2 * W], o_psum[b][:, W : 2 * W],
                    add=bias_sb[0:H2, :],
                )
                nc.sync.dma_start(
                    out=out_ap[0 : H2 // 2, b, :, :].rearrange(
                        "h p w -> h (p w)"
                    ),
                    in_=o_sb[b][0 : H2 // 2, :],
                )
                nc.scalar.dma_start(
                    out=out_ap[H2 // 2 : H2, b, :, :].rearrange(
                        "h p w -> h (p w)"
                    ),
                    in_=o_sb[b][H2 // 2 : H2, :],
                )

    nc.compile()
    return nc


def _get_program():
    global _PROGRAM
    if _PROGRAM is None:
        _PROGRAM = _build_program()
    return _PROGRAM


def _build_consts(K, bias, lambda_c, lambda_a):
    import ml_dtypes

    K = np.asarray(K, np.float32)
    lc = float(np.asarray(lambda_c))
    la = float(np.asarray(lambda_a))
    a = (lc + la) * K.sum(axis=0)  # column sums -> horizontal taps
    b = (la - lc) * K.sum(axis=1)  # row sums -> vertical taps
    eye = np.eye(H, dtype=np.float32)
    up = np.eye(H, k=1, dtype=np.float32)
    dn = np.eye(H, k=-1, dtype=np.float32)
    T = eye + up + dn
    A = a[1] * eye + a[0] * up + a[2] * dn
    Bm = b[1] * eye + b[2] * up + b[0] * dn
    cm = np.concatenate([A, T, Bm.T, eye], axis=1)
    cm16 = cm.astype(ml_dtypes.bfloat16)
    bias_col = np.full(
        (H, 1), np.asarray(bias, np.float32).reshape(-1)[0], np.float32
    )
    bias_bits = bias_col.view(np.uint16).view(ml_dtypes.bfloat16)  # [H, 2]
    return np.concatenate([cm16, bias_bits], axis=1)


def kernel(x, K, bias, lambda_c, lambda_a, _trace=False):
    global LAST_RESULTS
    import ml_dtypes
    from concourse.bass_utils import run_bass_kernel_spmd

    x = np.asarray(x, np.float32)
    cmb = _build_consts(K, bias, lambda_c, lambda_a)
    nc = _get_program()

    in_maps = []
    for core in range(N_CORES):
        shard = x[core * B_LOC : (core + 1) * B_LOC]  # [B_LOC, C, H, W]
        shard_t = shard.transpose(2, 0, 1, 3).astype(ml_dtypes.bfloat16)
        # fused per-partition rows: [cmat | batch1 | batch0]
        xc = np.concatenate(
            [
                cmb,
                shard_t[:, 1].reshape(H, C * W),
                shard_t[:, 0].reshape(H, C * W),
            ],
            axis=1,
        )
        in_maps.append({"xc": np.ascontiguousarray(xc)})

    res = run_bass_kernel_spmd(
        nc, in_maps, core_ids=list(range(N_CORES)), trace=_trace
    )
    LAST_RESULTS = res
    # per-core outputs are 2x-packed [H/2, B_LOC, 2, W]: row h = 2*p + par
    out = np.concatenate(
        [
            r["out"].transpose(1, 0, 2, 3).reshape(B_LOC, H, W)
            for r in res.results
        ],
        axis=0,
    )
    return out.reshape(B, 1, H, W).astype(np.float32, copy=False)



# revision 4
# speedup vs baseline: 1.1516x; 1.1516x over previous
"""CommutatorConv2d kernel for Trainium2 (Bass/Tile), 8-core data-parallel.

Math: the reference's commutator/anticommutator conv reduces exactly to a
single-channel 3x3 conv on the channel-summed input xs = x.sum(axis=1).
Writing the conv's horizontal taps as shifted copies and folding them into
the vertical band matrices gives a SINGLE matmul stage:

    out[b] = V0 @ shiftR(xs) + V1 @ xs + V2 @ shiftL(xs) + bias
    V_k = a[k]*T + Bm   (T tridiagonal-ones, Bm tridiagonal from K row
                         sums, a[k] from K column sums)

so the tail after streaming is: fold-finish -> 3 PSUM-accumulated matmuls
-> bias evacuation -> store.  (v5 needed two chained matmul stages with a
mid evac+cast.)

v6 layout/schedule (all bf16 in, f32 out):
- Input per core: [cmat(514 cols) | 4 PE pieces | 4 DVE pieces], 8ch
  (1024-col, 2KB-row) pieces.  cmat = [V0T|V1T|V2T|I|bias-bits].
- Batch 1 is streamed first on BOTH queues so its whole compute tail
  (reduce -> V matmuls -> bias -> store) hides under batch 0's streaming.
- Channel reduction split: PE identity-folds 16ch/batch into PSUM
  (fp32 accumulate), DVE trees 16ch/batch; a single tensor_reduce
  collapses the PSUM 4-partial stack, two adds merge in the tree results,
  writing xs into a zero-edged [128,130] pad buffer whose shifted column
  slices feed the three V matmuls directly.
"""

import numpy as np

B, C, H, W = 16, 32, 128, 128
N_CORES = 8
B_LOC = B // N_CORES

CMCOLS = 4 * W + 2
PIECE_CH = 8                       # channels per DMA piece
PIECE_COLS = PIECE_CH * W
N_JUNK = 8

_PROGRAM = None
LAST_RESULTS = None


def _build_program():
    import concourse.mybir as mybir
    from concourse import bacc
    from concourse.bass import MemorySpace
    from concourse.tile import TileContext

    bf16 = mybir.dt.bfloat16
    f32 = mybir.dt.float32
    nc = bacc.Bacc(
        "TRN2", target_bir_lowering=False, debug=False, num_devices=N_CORES
    )

    ncols = CMCOLS + 2 * C * W
    xc_dram = nc.dram_tensor("xc", (H, ncols), bf16, kind="ExternalInput")
    out_dram = nc.dram_tensor("out", (B_LOC, H, W), f32, kind="ExternalOutput")

    xc_ap = xc_dram.ap()
    out_ap = out_dram.ap()

    # column layout: [cmat | PE pieces s1..s4 | DVE pieces t1..t4]
    def pe_col(i):  # s_{i+1}
        return CMCOLS + i * PIECE_COLS

    def dve_col(i):  # t_{i+1}
        return CMCOLS + (4 + i) * PIECE_COLS

    with TileContext(nc) as tc:
        with (
            tc.tile_pool(name="xpool", bufs=1) as xpool,
            tc.tile_pool(name="spool", bufs=1) as spool,
            tc.tile_pool(name="psum", bufs=1, space=MemorySpace.PSUM) as ppool,
        ):
            # PE warmup scratch + zero-edged pad buffers (gpsimd, off-path)
            scratch = spool.tile([H, 5 * W], bf16, tag="scratch")
            nc.gpsimd.memset(scratch, 0.0)
            xsp = {}
            for b in (1, 0):
                t = spool.tile([H, W + 2], bf16, tag=f"xsp{b}")
                nc.gpsimd.memset(t, 0.0)
                xsp[b] = t

            # ---- input DMAs ----
            # sync: cmat then PE pieces (b1 first); scalar: DVE pieces
            cm_sb = xpool.tile([H, CMCOLS], bf16, tag="cmat")
            nc.sync.dma_start(out=cm_sb, in_=xc_ap[:, 0:CMCOLS])
            i_sb = cm_sb[:, 3 * W : 4 * W]
            bias_sb = cm_sb[:, 4 * W : 4 * W + 2].bitcast(f32)

            pe_t, dve_t = {}, {}
            for i in range(4):
                xq = xpool.tile([H, PIECE_COLS], bf16, tag=f"pe{i}")
                nc.sync.dma_start(
                    out=xq, in_=xc_ap[:, pe_col(i) : pe_col(i) + PIECE_COLS]
                )
                pe_t[i] = xq
            for i in range(4):
                xq = xpool.tile([H, PIECE_COLS], bf16, tag=f"dve{i}")
                nc.scalar.dma_start(
                    out=xq, in_=xc_ap[:, dve_col(i) : dve_col(i) + PIECE_COLS]
                )
                dve_t[i] = xq

            # ---- PE: junk warmup, then folds/V-matmuls ----
            junk_psum = ppool.tile([H, 4 * W], f32, tag="junk")
            for _ in range(N_JUNK):
                nc.tensor.matmul(
                    junk_psum,
                    scratch[:, 0:W],
                    scratch[:, W : 5 * W],
                    start=True,
                    stop=True,
                    skip_group_check=True,
                )

            fold_psum = {
                1: ppool.tile([H, 4 * W], f32, name="fp1", tag="fp1"),
                0: ppool.tile([H, 4 * W], f32, name="fp0", tag="fp0"),
            }
            o_psum = {
                1: ppool.tile([H, W], f32, name="op1", tag="op1"),
                0: ppool.tile([H, W], f32, name="op0", tag="op0"),
            }

            def tree(p):
                # [128,1024] -> [128,128] in place, result at p[:, 0:W]
                n = PIECE_COLS
                while n > W:
                    n //= 2
                    nc.vector.tensor_add(p[:, :n], p[:, :n], p[:, n : 2 * n])

            # program order per batch = dataflow order; batch 1 first so
            # its whole tail hides under batch 0's streaming
            for b, pe_pieces, dve_pieces in (
                (1, (pe_t[0], pe_t[1]), (dve_t[0], dve_t[1])),
                (0, (pe_t[2], pe_t[3]), (dve_t[2], dve_t[3])),
            ):
                # PE: identity-fold 16ch into 4 PSUM partials
                for pi, p in enumerate(pe_pieces):
                    for c in range(2):
                        nc.tensor.matmul(
                            fold_psum[b],
                            i_sb,
                            p[:, c * 4 * W : (c + 1) * 4 * W],
                            start=(pi == 0 and c == 0),
                            stop=(pi == 1 and c == 1),
                            skip_group_check=True,
                        )
                # DVE: tree 16ch, merge with PSUM partials into xsp
                ta, tb = dve_pieces
                tree(ta)
                tree(tb)
                nc.vector.tensor_add(ta[:, 0:W], ta[:, 0:W], tb[:, 0:W])
                tmp = spool.tile([H, W], bf16, name=f"tmp{b}", tag=f"tmp{b}")
                with nc.allow_low_precision("bf16 partials; gate is 2e-2"):
                    nc.vector.tensor_reduce(
                        tmp,
                        fold_psum[b][:, 0 : 4 * W].rearrange(
                            "p (j w) -> p w j", j=4
                        ),
                        axis=mybir.AxisListType.X,
                        op=mybir.AluOpType.add,
                    )
                nc.vector.tensor_add(xsp[b][:, 1 : W + 1], ta[:, 0:W], tmp)
                # PE: the three shifted V matmuls
                for k in range(3):
                    nc.tensor.matmul(
                        o_psum[b],
                        cm_sb[:, k * W : (k + 1) * W],
                        xsp[b][:, k : k + W],
                        start=(k == 0),
                        stop=(k == 2),
                        skip_group_check=True,
                    )
                # ACT: bias evac; store halves split across both queues
                osb = spool.tile([H, W], f32, name=f"o{b}", tag=f"o{b}")
                nc.scalar.add(osb, o_psum[b], add=bias_sb)
                nc.sync.dma_start(
                    out=out_ap[b, 0 : H // 2, :], in_=osb[0 : H // 2, :]
                )
                nc.scalar.dma_start(
                    out=out_ap[b, H // 2 : H, :], in_=osb[H // 2 : H, :]
                )

    nc.compile()
    return nc


def _get_program():
    global _PROGRAM
    if _PROGRAM is None:
        _PROGRAM = _build_program()
    return _PROGRAM


def _build_consts(K, bias, lambda_c, lambda_a):
    import ml_dtypes

    K = np.asarray(K, np.float32)
    lc = float(np.asarray(lambda_c))
    la = float(np.asarray(lambda_a))
    a = (lc + la) * K.sum(axis=0)  # column sums -> horizontal taps
    b = (la - lc) * K.sum(axis=1)  # row sums -> vertical taps
    eye = np.eye(H, dtype=np.float32)
    up = np.eye(H, k=1, dtype=np.float32)
    dn = np.eye(H, k=-1, dtype=np.float32)
    T = eye + up + dn
    Bm = b[1] * eye + b[2] * up + b[0] * dn
    vs = [np.ascontiguousarray((a[k] * T + Bm).T) for k in range(3)]
    cm = np.concatenate(vs + [eye], axis=1)
    cm16 = cm.astype(ml_dtypes.bfloat16)
    bias_col = np.full(
        (H, 1), np.asarray(bias, np.float32).reshape(-1)[0], np.float32
    )
    bias_bits = bias_col.view(np.uint16).view(ml_dtypes.bfloat16)  # [H, 2]
    return np.concatenate([cm16, bias_bits], axis=1)


def kernel(x, K, bias, lambda_c, lambda_a, _trace=False):
    global LAST_RESULTS
    import ml_dtypes
    from concourse.bass_utils import run_bass_kernel_spmd

    x = np.asarray(x, np.float32)
    cmb = _build_consts(K, bias, lambda_c, lambda_a)
    nc = _get_program()

    in_maps = []
    for core in range(N_CORES):
        shard = x[core * B_LOC : (core + 1) * B_LOC]  # [2, C, H, W]
        st = shard.transpose(2, 0, 1, 3).astype(ml_dtypes.bfloat16)  # [H,2,C,W]
        # PE pieces: b1c0-7, b1c8-15, b0c0-7, b0c8-15
        # DVE pieces: b1c16-23, b1c24-31, b0c16-23, b0c24-31
        blocks = [cmb]
        for b, c0 in ((1, 0), (1, 8), (0, 0), (0, 8)):
            blocks.append(st[:, b, c0 : c0 + 8].reshape(H, PIECE_COLS))
        for b, c0 in ((1, 16), (1, 24), (0, 16), (0, 24)):
            blocks.append(st[:, b, c0 : c0 + 8].reshape(H, PIECE_COLS))
        xc = np.concatenate(blocks, axis=1)
        in_maps.append({"xc": np.ascontiguousarray(xc)})

    res = run_bass_kernel_spmd(
        nc, in_maps, core_ids=list(range(N_CORES)), trace=_trace
    )
    LAST_RESULTS = res
    out = np.concatenate([r["out"] for r in res.results], axis=0)
    return out.reshape(B, 1, H, W).astype(np.float32, copy=False)
